# revision 43
# baseline (speedup 1.0000x reference)
"""Trainium2 Bass kernel for nn_BaselineWormholeRouter.

Computes, per batch b over x[b, 1:, :]:
  q = l2norm(x @ Wq.T + bq); k = l2norm(x @ Wk.T + bk); v = x @ Wv.T + bv
  scores = q @ k.T with diag masked to -1e9
  routes, topk = top8(scores / 0.1); weights = softmax(topk)
  features = sum_k weights[k] * v[routes[k]]

Sharding: 8 cores = (batch b in 0..3) x (query-row half h in 0..1).
Each core computes q/k/v for its own 1024 rows, pair-exchanges k^T and v
via AllGather over core pairs (0,1),(2,3),(4,6),(6,7), then does the
full scores row-block + top8 + masked-softmax features locally.

Precision: Q/K projections and the scores matmul use a bf16 hi/lo
split (x ~ hi + lo, 3 accumulation passes: hh + hl + lh) giving
~2e-7-relative score accuracy at bf16 matmul throughput -- plain fp32r
or bf16 would swap too many near-tied top-8 entries (routes are
rank-exact vs the fp32 reference except a handful of ties within
~2e-7). The norm uses IEEE reciprocal + Newton-refined sqrt. V and the
features matmul run in plain bf16 (features gate is loose).
"""
import os
import sys

import numpy as np

sys.path.insert(0, "/opt/trn_rl_repo")

import concourse.bass as bass  # noqa: E402
import concourse.tile as tile  # noqa: E402
from concourse import bacc, mybir  # noqa: E402
from concourse.bass import ts  # noqa: E402
from concourse.bass_utils import run_bass_kernel_spmd  # noqa: E402
from concourse.masks import make_identity  # noqa: E402

F32 = mybir.dt.float32
BF16 = mybir.dt.bfloat16
U32 = mybir.dt.uint32

N_CORES = 8
B, P1, D = 4, 2049, 1024
P = P1 - 1          # 2048 rows after skip_first
H = P // 2          # 1024 rows per core
NEG_INF = -1e9
TEMP_INV = 10.0     # 1/temperature
RG = [[0, 1], [2, 3], [4, 5], [6, 7]]

_CACHE = {}


def _build():
    nc = bacc.Bacc("TRN2", target_bir_lowering=False, debug=False,
                   num_devices=N_CORES)

    # ---- DRAM I/O ----
    xq_e = nc.dram_tensor("xq", [H, D], F32, kind="ExternalInput").ap()
    wq_e = nc.dram_tensor("wq", [D, D], F32, kind="ExternalInput").ap()
    wk_e = nc.dram_tensor("wk", [D, D], F32, kind="ExternalInput").ap()
    wv_e = nc.dram_tensor("wv", [D, D], F32, kind="ExternalInput").ap()
    bq_e = nc.dram_tensor("bq", [1, D], F32, kind="ExternalInput").ap()
    bk_e = nc.dram_tensor("bk", [1, D], F32, kind="ExternalInput").ap()
    bv_e = nc.dram_tensor("bv", [1, D], F32, kind="ExternalInput").ap()
    rid_e = nc.dram_tensor("row_ids", [H], F32, kind="ExternalInput").ap()

    routes_e = nc.dram_tensor("routes", [H, 8], U32, kind="ExternalOutput").ap()
    weights_e = nc.dram_tensor("weights", [H, 8], F32, kind="ExternalOutput").ap()
    features_e = nc.dram_tensor("features", [H, D], F32, kind="ExternalOutput").ap()

    kt_bounce = nc.dram_tensor("kt_bounce", [2, D, H], BF16).ap()
    v_bounce = nc.dram_tensor("v_bounce", [H, D], BF16).ap()
    kt_gather_hi = nc.dram_tensor("kt_gather_hi", [2, D, H], BF16).ap()
    kt_gather_lo = nc.dram_tensor("kt_gather_lo", [2, D, H], BF16).ap()
    v_gather = nc.dram_tensor("v_gather", [2, H, D], BF16).ap()

    NOC = H // 128   # 8 row chunks per core
    NDC = D // 128   # 8 contraction chunks

    with tile.TileContext(nc) as tc:
        from contextlib import ExitStack
        with ExitStack() as top:
            cpool = top.enter_context(tc.tile_pool(name="consts", bufs=1))
            small = top.enter_context(tc.tile_pool(name="small", bufs=6))
            ktfp = top.enter_context(tc.tile_pool(name="ktf_pool", bufs=1))
            psA = top.enter_context(tc.tile_pool(name="psA", bufs=4, space="PSUM"))
            psT = top.enter_context(tc.tile_pool(name="psT", bufs=2, space="PSUM"))
            xq_scope = tc.tile_pool(name="xqt_pool", bufs=1)
            xq_pool = xq_scope.__enter__()

            # ---- constants ----
            ident = cpool.tile([128, 128], F32, tag="ident")
            make_identity(nc, ident[:])
            ident_bf = cpool.tile([128, 128], BF16, tag="ident_bf")
            nc.vector.tensor_copy(ident_bf[:], ident[:])
            iota_f = cpool.tile([128, P], F32, tag="iota")
            nc.gpsimd.iota(iota_f[:], pattern=[[1, P]], base=0,
                           channel_multiplier=0,
                           allow_small_or_imprecise_dtypes=True)
            warm = cpool.tile([128, 512], BF16, tag="warm")
            nc.vector.memset(warm[:], 0.0)
            for wi in range(16):
                pwm = psA.tile([128, 512], F32, tag="mm")
                nc.tensor.matmul(pwm[:], warm[:, 0:128], warm[:],
                                 start=True, stop=True)
                nc.scalar.copy(warm[0:1, 0:1], pwm[0:1, 0:1])

            rid_sb = cpool.tile([128, NOC], F32, tag="rid")
            nc.sync.dma_start(rid_sb[:], rid_e.rearrange("(o p) -> p o", p=128))
            ones_r = cpool.tile([1, 128], F32, tag="ones")
            nc.vector.memset(ones_r[:], 1.0)

            # bias rows as bf16 hi/lo pairs; added inside each matmul
            # accumulation via a K=2 ones matmul.
            ones2 = cpool.tile([2, 128], BF16, tag="ones2")
            nc.vector.memset(ones2[:], 1.0)
            bias_bc = {}
            with tc.tile_pool(name="browp", bufs=2) as browp:
                for nm, be in (("q", bq_e), ("k", bk_e), ("v", bv_e)):
                    brow = browp.tile([1, D], F32, tag="brow")
                    nc.sync.dma_start(brow[:], be)
                    bh = browp.tile([1, D], BF16, tag="bh")
                    nc.scalar.copy(bh[:], brow[:])
                    bl = browp.tile([1, D], BF16, tag="bl")
                    nc.vector.tensor_sub(bl[:], brow[:], bh[:])
                    b2 = cpool.tile([2, D], BF16, tag=f"bias_{nm}")
                    nc.sync.dma_start(b2[0:1, :], bh[:])
                    nc.sync.dma_start(b2[1:2, :], bl[:])
                    bias_bc[nm] = b2

            def copy_split(dst_hi, dst_lo, psum):
                """PSUM f32 -> bf16 hi + bf16 residual lo."""
                nc.scalar.copy(dst_hi, psum)
                nc.vector.tensor_sub(dst_lo, psum, dst_hi)

            wq_scope = tc.tile_pool(name="wt_wq", bufs=1)
            wqrp = wq_scope.__enter__()
            wv_scope = tc.tile_pool(name="wt_wv", bufs=1)
            wvrp = wv_scope.__enter__()

            # ---- transpose own x rows -> XQT hi/lo [128, dc, row] ----
            xqt_hi = xq_pool.tile([128, NDC, H], BF16, tag="xqt_hi")
            xqt_lo = xq_pool.tile([128, NDC, H], BF16, tag="xqt_lo")
            with tc.tile_pool(name="xin", bufs=2) as xin:
                for oc in range(NOC):
                    xc = xin.tile([128, D], F32, tag="xc")
                    nc.sync.dma_start(xc[:], xq_e[ts(oc, 128), :])
                    for dg in range(2):
                        pt = psT.tile([128, 512], F32, tag="pt")
                        for i in range(4):
                            dc = dg * 4 + i
                            nc.tensor.transpose(pt[:, ts(i, 128)],
                                                xc[:, ts(dc, 128)], ident[:])
                        copy_split(xqt_hi[:, dg * 4:dg * 4 + 4, ts(oc, 128)],
                                   xqt_lo[:, dg * 4:dg * 4 + 4, ts(oc, 128)],
                                   pt[:])

            def transpose_weight(w_ext, out_dtype, tag, wt_pool, split=False):
                """DRAM W[d, d'] -> SBUF WT[128(d'p), d'c, d] (= W.T).
                split=True returns (hi, lo) bf16 pair."""
                if split:
                    wt_hi = wt_pool.tile([128, NDC, D], BF16, tag=f"{tag}_hi")
                    wt_lo = wt_pool.tile([128, NDC, D], BF16, tag=f"{tag}_lo")
                else:
                    wt = wt_pool.tile([128, NDC, D], out_dtype, tag=tag)
                with tc.tile_pool(name=f"win_{tag}", bufs=4) as win:
                    for oc in range(NDC):
                        wc = win.tile([128, D], F32, tag="wc")
                        nc.sync.dma_start(wc[:], w_ext[ts(oc, 128), :])
                        for dg in range(2):
                            pt = psT.tile([128, 512], F32, tag="pt")
                            for i in range(4):
                                dc = dg * 4 + i
                                nc.tensor.transpose(pt[:, ts(i, 128)],
                                                    wc[:, ts(dc, 128)], ident[:])
                            sl = (slice(None), slice(dg * 4, dg * 4 + 4),
                                  ts(oc, 128))
                            if split:
                                copy_split(wt_hi[sl], wt_lo[sl], pt[:])
                            else:
                                nc.scalar.copy(wt[sl], pt[:])
                return (wt_hi, wt_lo) if split else wt

            def rsqrt_newton(ssq):
                """[128,1] fp32 1/sqrt(ssq), fp32-accurate via 2 Newton steps."""
                s1 = small.tile([128, 1], F32, tag="nw_s1")
                nc.scalar.activation(s1[:], ssq[:],
                                     mybir.ActivationFunctionType.Sqrt)
                z = small.tile([128, 1], F32, tag="nw_z")
                nc.vector.reciprocal(z[:], s1[:])
                for _ in range(2):
                    t = small.tile([128, 1], F32, tag="nw_t")
                    nc.vector.tensor_mul(t[:], z[:], z[:])
                    nc.vector.tensor_mul(t[:], t[:], ssq[:])
                    nc.vector.tensor_scalar(t[:], t[:], -0.5, 1.5,
                                            op0=mybir.AluOpType.mult,
                                            op1=mybir.AluOpType.add)
                    nc.vector.tensor_mul(z[:], z[:], t[:])
                return z

            def qk_phase(w_ext, bias, out_hi, out_lo, wtag, pre=None,
                         dram_sink=None, mid_cb=None):
                """Project own rows with W (hi/lo 3-pass bf16), l2-normalize,
                transpose into split out_hi/out_lo [128, dc, row]."""
                with tc.tile_pool(name=f"wt_{wtag}", bufs=1) as wtp, \
                     tc.tile_pool(name=f"ph_{wtag}", bufs=2) as ph:
                    if pre is None:
                        wt_hi, wt_lo = transpose_weight(w_ext, BF16, wtag, wtp,
                                                        split=True)
                    else:
                        wt_hi, wt_lo = pre
                    for oc in range(NOC):
                        if oc == 4 and mid_cb is not None:
                            mid_cb()
                        qh = ph.tile([128, D], F32, tag="qh")
                        pqs = []
                        sqa = []
                        for db in range(2):
                            pq = psA.tile([128, 512], F32, tag="mm")
                            pqs.append(pq)
                            for dc in range(NDC):
                                for li, (lh, rh) in enumerate((
                                        (xqt_hi, wt_hi), (xqt_hi, wt_lo),
                                        (xqt_lo, wt_hi))):
                                    nc.tensor.matmul(
                                        pq[:], lh[:, dc, ts(oc, 128)],
                                        rh[:, dc, ts(db, 512)],
                                        start=(dc == 0 and li == 0),
                                        stop=False)
                            nc.tensor.matmul(pq[:], ones2[:],
                                             bias[:, ts(db, 512)],
                                             start=False, stop=True)
                            sq = small.tile([128, 1], F32, tag=f"ssq{db}")
                            sqa.append(sq)
                            nc.scalar.activation(
                                qh[:, ts(db, 512)], pq[:],
                                mybir.ActivationFunctionType.Square,
                                accum_out=sq[:])
                        ssq = small.tile([128, 1], F32, tag="ssq")
                        nc.vector.tensor_add(ssq[:], sqa[0][:], sqa[1][:])
                        z = rsqrt_newton(ssq)
                        for db in range(2):
                            nc.scalar.activation(
                                qh[:, ts(db, 512)], pqs[db][:],
                                mybir.ActivationFunctionType.Copy,
                                scale=z[:])
                        for dg in range(2):
                            pt = psT.tile([128, 512], F32, tag="pt")
                            for i in range(4):
                                dc = dg * 4 + i
                                nc.tensor.transpose(pt[:, ts(i, 128)],
                                                    qh[:, ts(dc, 128)], ident[:])
                            if dram_sink is None:
                                sl = (slice(None), slice(dg * 4, dg * 4 + 4),
                                      ts(oc, 128))
                                copy_split(out_hi[sl], out_lo[sl], pt[:])
                            else:
                                st_hi = ph.tile([128, 4, 128], BF16, tag="st_hi")
                                st_lo = ph.tile([128, 4, 128], BF16, tag="st_lo")
                                copy_split(st_hi[:], st_lo[:], pt[:])
                                for hl, st in ((0, st_hi), (1, st_lo)):
                                    nc.gpsimd.dma_start(
                                        dram_sink[hl, ts(dg, 512),
                                                  ts(oc, 128)].rearrange(
                                            "(o p) j -> p o j", p=128),
                                        st[:])

            ktf_hi = ktfp.tile([128, NDC, P], BF16, tag="ktf_hi")
            wpre = {}

            def build_wq_wv():
                wpre["wq"] = transpose_weight(wq_e, BF16, "wq", wqrp, split=True)
                wpre["wv"] = transpose_weight(wv_e, BF16, "wv", wvrp)

            PHASES = os.environ.get("KERNEL_PHASES", "full")
            # ---- K first (feeds the collective), then Q, then V ----
            qk_phase(wk_e, bias_bc["k"], None, None, "wk", dram_sink=kt_bounce,
                     mid_cb=build_wq_wv)
            wqt_hi, wqt_lo = wpre["wq"]
            wvt = wpre["wv"]
            ktflp = top.enter_context(
                tc.tile_pool(name="ktf_lo_pool", bufs=1, side="right"))
            ktf_lo = ktflp.tile([128, NDC, P], BF16, tag="ktf_lo")
            if PHASES != "qkv":
                nc.gpsimd.collective_compute(
                    "AllGather", mybir.AluOpType.bypass, replica_groups=RG,
                    ins=[kt_bounce[0]], outs=[kt_gather_hi[:]])
                nc.gpsimd.collective_compute(
                    "AllGather", mybir.AluOpType.bypass, replica_groups=RG,
                    ins=[kt_bounce[1]], outs=[kt_gather_lo[:]])

            # ---- V in bf16 ----
            with tc.tile_pool(name="phv", bufs=2) as phv:
                for oc in range(NOC):
                    vsb = phv.tile([128, D], BF16, tag="vsb")
                    for db in range(2):
                        pv = psA.tile([128, 512], F32, tag="mm")
                        for dc in range(NDC):
                            nc.tensor.matmul(pv[:], xqt_hi[:, dc, ts(oc, 128)],
                                             wvt[:, dc, ts(db, 512)],
                                             start=(dc == 0), stop=False)
                        nc.tensor.matmul(pv[:], ones2[:],
                                         bias_bc["v"][:, ts(db, 512)],
                                         start=False, stop=True)
                        nc.scalar.copy(vsb[:, ts(db, 512)], pv[:])
                    nc.gpsimd.dma_start(v_bounce[ts(oc, 128), :], vsb[:])
            if PHASES != "qkv":
                nc.gpsimd.collective_compute(
                    "AllGather", mybir.AluOpType.bypass, replica_groups=RG,
                    ins=[v_bounce[:]], outs=[v_gather[:]])

            qtp = top.enter_context(tc.tile_pool(name="qt_pool", bufs=1, side="right"))
            qt_hi = qtp.tile([128, NDC, H], BF16, tag="qt_hi")
            qt_lo = qtp.tile([128, NDC, H], BF16, tag="qt_lo")
            qk_phase(wq_e, bias_bc["q"], qt_hi, qt_lo, "wq", pre=(wqt_hi, wqt_lo))

            # ---- load gathered K^T and V ----
            wv_scope.__exit__(None, None, None)
            wq_scope.__exit__(None, None, None)
            xq_scope.__exit__(None, None, None)
            bigp = top.enter_context(tc.tile_pool(name="big_pool", bufs=1))
            vf = bigp.tile([128, 2 * NOC, D], BF16, tag="vf")
            if PHASES != "qkv":
                for g in range(2):
                    nc.sync.dma_start(
                        ktf_hi[:, :, g * H:(g + 1) * H],
                        kt_gather_hi[g].rearrange("(o p) j -> p o j", p=128))
                for g in range(2):
                    nc.sync.dma_start(
                        ktf_lo[:, :, g * H:(g + 1) * H],
                        kt_gather_lo[g].rearrange("(o p) j -> p o j", p=128))
                for g in range(2):
                    nc.gpsimd.dma_start(
                        vf[:, g * NOC:(g + 1) * NOC, :],
                        v_gather[g].rearrange("(o p) d -> p o d", p=128))

            # ---- fused scores / top8 / masked-softmax / features ----
            run_ph6 = PHASES not in ("qkv", "cc")
            with tc.tile_pool(name="sc", bufs=2) as sc, \
                 tc.tile_pool(name="dmp", bufs=1) as dmp, \
                 tc.tile_pool(name="wtrp", bufs=4) as wtrp:
                idx_all = sc.tile([128, NOC, 8], U32, tag="idx_all")
                w_all = sc.tile([128, NOC, 8], F32, tag="w_all")

                def stage_scores(pc):
                    S = sc.tile([128, P], F32, tag="S")
                    # diagonal mask, built before the matmuls finish:
                    # dm = -1e9 where j == global row id else 0
                    dm = dmp.tile([128, P], BF16, tag="dm")
                    nc.vector.tensor_scalar(dm[:], iota_f[:],
                                            rid_sb[:, pc:pc + 1], NEG_INF,
                                            op0=mybir.AluOpType.is_equal,
                                            op1=mybir.AluOpType.mult)
                    for jb in range(4):
                        ps = psA.tile([128, 512], F32, tag="mm")
                        for dc in range(NDC):
                            for li, (lh, rh) in enumerate((
                                    (qt_hi, ktf_hi), (qt_hi, ktf_lo),
                                    (qt_lo, ktf_hi))):
                                nc.tensor.matmul(
                                    ps[:], lh[:, dc, ts(pc, 128)],
                                    rh[:, dc, ts(jb, 512)],
                                    start=(dc == 0 and li == 0),
                                    stop=(dc == NDC - 1 and li == 2))
                        nc.vector.tensor_add(S[:, ts(jb, 512)], ps[:],
                                             dm[:, ts(jb, 512)])
                    # top-8 values + indices
                    vals = small.tile([128, 8], F32, tag="vals")
                    nc.vector.max(out=vals[:], in_=S[:])
                    nc.vector.max_index(out=idx_all[:, pc, :], in_max=vals[:],
                                        in_values=S[:])
                    # softmax weights over the top-8
                    negmax = small.tile([128, 1], F32, tag="negmax")
                    nc.vector.tensor_scalar_mul(negmax[:], vals[:, 0:1], -TEMP_INV)
                    e8 = small.tile([128, 8], F32, tag="e8")
                    nc.scalar.activation(e8[:], vals[:],
                                         mybir.ActivationFunctionType.Exp,
                                         bias=negmax[:], scale=TEMP_INV)
                    s8 = small.tile([128, 1], F32, tag="s8")
                    nc.vector.tensor_reduce(s8[:], e8[:], axis=mybir.AxisListType.X,
                                            op=mybir.AluOpType.add)
                    r8 = small.tile([128, 1], F32, tag="r8")
                    nc.vector.reciprocal(r8[:], s8[:])
                    nc.vector.tensor_scalar_mul(w_all[:, pc, :], e8[:], r8[:])
                    # masked exp row -> sparse weight matrix (bf16)
                    eb = sc.tile([128, P], BF16, tag="eb")
                    nc.scalar.activation(eb[:], S[:],
                                         mybir.ActivationFunctionType.Exp,
                                         bias=negmax[:], scale=TEMP_INV)
                    nc.vector.scalar_tensor_tensor(
                        out=eb[:], in0=S[:], scalar=vals[:, 7:8], in1=eb[:],
                        op0=mybir.AluOpType.is_ge, op1=mybir.AluOpType.mult)
                    # transpose W row-block
                    wtr = wtrp.tile([128, 2 * NOC, 128], BF16, tag="wtr")
                    for jg in range(2):
                        pw = psT.tile([128, 1024], BF16, tag="pw")
                        for i in range(8):
                            jc = jg * 8 + i
                            nc.tensor.transpose(pw[:, ts(i, 128)],
                                                eb[:, ts(jc, 128)], ident_bf[:])
                        nc.scalar.copy(wtr[:, jg * 8:jg * 8 + 8, :], pw[:])
                    return wtr, r8

                def stage_features(pc, wtr, r8):
                    feat = sc.tile([128, D], F32, tag="feat")
                    for db in range(2):
                        pf = psA.tile([128, 512], F32, tag="mm")
                        for jc in range(2 * NOC):
                            nc.tensor.matmul(pf[:], wtr[:, jc, :],
                                             vf[:, jc, ts(db, 512)],
                                             start=(jc == 0),
                                             stop=(jc == 2 * NOC - 1))
                        nc.scalar.activation(feat[:, ts(db, 512)], pf[:],
                                             mybir.ActivationFunctionType.Copy,
                                             scale=r8[:])
                    nc.sync.dma_start(features_e[ts(pc, 128), :], feat[:])

                SKEW = 3
                pend = []
                for pc in range(NOC if run_ph6 else 0):
                    pend.append((pc, stage_scores(pc)))
                    if len(pend) > SKEW:
                        fpc, args = pend.pop(0)
                        stage_features(fpc, *args)
                for fpc, args in pend:
                    stage_features(fpc, *args)
                if run_ph6:
                    nc.sync.dma_start(routes_e.rearrange("(o p) k -> p o k", p=128),
                                      idx_all[:])
                    nc.sync.dma_start(weights_e.rearrange("(o p) k -> p o k", p=128),
                                      w_all[:])

    nc.compile()
    return nc


def _get_nc():
    if "nc" not in _CACHE:
        _CACHE["nc"] = _build()
    return _CACHE["nc"]


def kernel(x, Wq, bq, Wk, bk, Wv, bv):
    x = np.ascontiguousarray(np.asarray(x, dtype=np.float32))
    Wq = np.ascontiguousarray(np.asarray(Wq, dtype=np.float32))
    Wk = np.ascontiguousarray(np.asarray(Wk, dtype=np.float32))
    Wv = np.ascontiguousarray(np.asarray(Wv, dtype=np.float32))
    bq = np.asarray(bq, dtype=np.float32).reshape(1, D)
    bk = np.asarray(bk, dtype=np.float32).reshape(1, D)
    bv = np.asarray(bv, dtype=np.float32).reshape(1, D)

    nc = _get_nc()
    in_maps = []
    for c in range(N_CORES):
        b, h = c // 2, c % 2
        xs = np.ascontiguousarray(x[b, 1 + h * H:1 + (h + 1) * H, :])
        rid = (h * H + np.arange(H)).astype(np.float32)
        in_maps.append({
            "xq": xs, "wq": Wq, "wk": Wk, "wv": Wv,
            "bq": bq, "bk": bk, "bv": bv, "row_ids": rid,
        })

    trace = bool(int(os.environ.get("BASS_KERNEL_TRACE", "0")))
    res = run_bass_kernel_spmd(nc, in_maps, core_ids=list(range(N_CORES)),
                               trace=trace)
    _CACHE["last_result"] = res

    routes = np.empty((B, P, 8), dtype=np.int32)
    weights = np.empty((B, P, 8), dtype=np.float32)
    features = np.empty((B, P, D), dtype=np.float32)
    for c in range(N_CORES):
        b, h = c // 2, c % 2
        sl = slice(h * H, (h + 1) * H)
        routes[b, sl] = res.results[c]["routes"].astype(np.int32)
        weights[b, sl] = res.results[c]["weights"]
        features[b, sl] = res.results[c]["features"]
    return routes, weights, features


# revision 44
# speedup vs baseline: 1.0141x; 1.0141x over previous
"""Trainium2 Bass kernel for nn_BaselineWormholeRouter.

Computes, per batch b over x[b, 1:, :]:
  q = l2norm(x @ Wq.T + bq); k = l2norm(x @ Wk.T + bk); v = x @ Wv.T + bv
  scores = q @ k.T with diag masked to -1e9
  routes, topk = top8(scores / 0.1); weights = softmax(topk)
  features = sum_k weights[k] * v[routes[k]]

Sharding: 8 cores = (batch b in 0..3) x (query-row half h in 0..1).
Each core computes q/k/v for its own 1024 rows, pair-exchanges k^T and v
via AllGather over core pairs (0,1),(2,3),(4,6),(6,7), then does the
full scores row-block + top8 + masked-softmax features locally.

Precision: Q/K projections and the scores matmul use a bf16 hi/lo
split (x ~ hi + lo, 3 accumulation passes: hh + hl + lh) giving
~2e-7-relative score accuracy at bf16 matmul throughput -- plain fp32r
or bf16 would swap too many near-tied top-8 entries (routes are
rank-exact vs the fp32 reference except a handful of ties within
~2e-7). The norm uses IEEE reciprocal + Newton-refined sqrt. V and the
features matmul run in plain bf16 (features gate is loose).
"""
import os
import sys

import numpy as np

sys.path.insert(0, "/opt/trn_rl_repo")

import concourse.bass as bass  # noqa: E402
import concourse.tile as tile  # noqa: E402
from concourse import bacc, mybir  # noqa: E402
from concourse.bass import ts  # noqa: E402
from concourse.bass_utils import run_bass_kernel_spmd  # noqa: E402
from concourse.masks import make_identity  # noqa: E402

F32 = mybir.dt.float32
BF16 = mybir.dt.bfloat16
U32 = mybir.dt.uint32

N_CORES = 8
B, P1, D = 4, 2049, 1024
P = P1 - 1          # 2048 rows after skip_first
H = P // 2          # 1024 rows per core
NEG_INF = -1e9
TEMP_INV = 10.0     # 1/temperature
RG = [[0, 1], [2, 3], [4, 5], [6, 7]]

_CACHE = {}


def _build():
    nc = bacc.Bacc("TRN2", target_bir_lowering=False, debug=False,
                   num_devices=N_CORES)

    # ---- DRAM I/O ----
    xq_e = nc.dram_tensor("xq", [H, D], F32, kind="ExternalInput").ap()
    wq_e = nc.dram_tensor("wq", [D, D], F32, kind="ExternalInput").ap()
    wk_e = nc.dram_tensor("wk", [D, D], F32, kind="ExternalInput").ap()
    wv_e = nc.dram_tensor("wv", [D, D], F32, kind="ExternalInput").ap()
    bq_e = nc.dram_tensor("bq", [1, D], F32, kind="ExternalInput").ap()
    bk_e = nc.dram_tensor("bk", [1, D], F32, kind="ExternalInput").ap()
    bv_e = nc.dram_tensor("bv", [1, D], F32, kind="ExternalInput").ap()
    rid_e = nc.dram_tensor("row_ids", [H], F32, kind="ExternalInput").ap()

    routes_e = nc.dram_tensor("routes", [H, 8], U32, kind="ExternalOutput").ap()
    weights_e = nc.dram_tensor("weights", [H, 8], F32, kind="ExternalOutput").ap()
    features_e = nc.dram_tensor("features", [H, D], F32, kind="ExternalOutput").ap()

    kt_bounce = nc.dram_tensor("kt_bounce", [2, D, H], BF16).ap()
    v_bounce = nc.dram_tensor("v_bounce", [H, D], BF16).ap()
    kt_gather_hi = nc.dram_tensor("kt_gather_hi", [2, D, H], BF16).ap()
    kt_gather_lo = nc.dram_tensor("kt_gather_lo", [2, D, H], BF16).ap()
    v_gather = nc.dram_tensor("v_gather", [2, H, D], BF16).ap()

    NOC = H // 128   # 8 row chunks per core
    NDC = D // 128   # 8 contraction chunks

    with tile.TileContext(nc) as tc:
        from contextlib import ExitStack
        with ExitStack() as top:
            cpool = top.enter_context(tc.tile_pool(name="consts", bufs=1))
            small = top.enter_context(tc.tile_pool(name="small", bufs=6))
            ktfp = top.enter_context(tc.tile_pool(name="ktf_pool", bufs=1))
            psA = top.enter_context(tc.tile_pool(name="psA", bufs=4, space="PSUM"))
            psT = top.enter_context(tc.tile_pool(name="psT", bufs=2, space="PSUM"))
            xq_scope = tc.tile_pool(name="xqt_pool", bufs=1)
            xq_pool = xq_scope.__enter__()

            # ---- constants ----
            ident = cpool.tile([128, 128], F32, tag="ident")
            make_identity(nc, ident[:])
            ident_bf = cpool.tile([128, 128], BF16, tag="ident_bf")
            nc.vector.tensor_copy(ident_bf[:], ident[:])
            iota_f = cpool.tile([128, P], F32, tag="iota")
            nc.gpsimd.iota(iota_f[:], pattern=[[1, P]], base=0,
                           channel_multiplier=0,
                           allow_small_or_imprecise_dtypes=True)
            warm = cpool.tile([128, 512], BF16, tag="warm")
            nc.vector.memset(warm[:], 0.0)
            for wi in range(16):
                pwm = psA.tile([128, 512], F32, tag="mm")
                nc.tensor.matmul(pwm[:], warm[:, 0:128], warm[:],
                                 start=True, stop=True)
                nc.scalar.copy(warm[0:1, 0:1], pwm[0:1, 0:1])

            rid_sb = cpool.tile([128, NOC], F32, tag="rid")
            nc.sync.dma_start(rid_sb[:], rid_e.rearrange("(o p) -> p o", p=128))
            ones_r = cpool.tile([1, 128], F32, tag="ones")
            nc.vector.memset(ones_r[:], 1.0)

            # bias rows as bf16 hi/lo pairs; added inside each matmul
            # accumulation via a K=2 ones matmul.
            ones2 = cpool.tile([2, 128], BF16, tag="ones2")
            nc.vector.memset(ones2[:], 1.0)
            bias_bc = {}
            with tc.tile_pool(name="browp", bufs=2) as browp:
                for nm, be in (("q", bq_e), ("k", bk_e), ("v", bv_e)):
                    brow = browp.tile([1, D], F32, tag="brow")
                    nc.sync.dma_start(brow[:], be)
                    bh = browp.tile([1, D], BF16, tag="bh")
                    nc.scalar.copy(bh[:], brow[:])
                    bl = browp.tile([1, D], BF16, tag="bl")
                    nc.vector.tensor_sub(bl[:], brow[:], bh[:])
                    b2 = cpool.tile([2, D], BF16, tag=f"bias_{nm}")
                    nc.sync.dma_start(b2[0:1, :], bh[:])
                    nc.sync.dma_start(b2[1:2, :], bl[:])
                    bias_bc[nm] = b2

            def copy_split(dst_hi, dst_lo, psum):
                """PSUM f32 -> bf16 hi + bf16 residual lo."""
                nc.scalar.copy(dst_hi, psum)
                nc.vector.tensor_sub(dst_lo, psum, dst_hi)

            wq_scope = tc.tile_pool(name="wt_wq", bufs=1)
            wqrp = wq_scope.__enter__()
            wv_scope = tc.tile_pool(name="wt_wv", bufs=1)
            wvrp = wv_scope.__enter__()

            # ---- interleaved build: XQT hi/lo and WkT hi/lo ----
            # x chunks and wk chunks stream on the same sync queue but gate
            # different transpose groups, so the PE always has one ready.
            wk_scope = tc.tile_pool(name="wt_wk_pre", bufs=1)
            wkpp = wk_scope.__enter__()
            xqt_hi = xq_pool.tile([128, NDC, H], BF16, tag="xqt_hi")
            xqt_lo = xq_pool.tile([128, NDC, H], BF16, tag="xqt_lo")
            wkt_hi = wkpp.tile([128, NDC, D], BF16, tag="wk_hi")
            wkt_lo = wkpp.tile([128, NDC, D], BF16, tag="wk_lo")
            with tc.tile_pool(name="xin", bufs=2) as xin, \
                 tc.tile_pool(name="wink", bufs=2) as wink:
                for oc in range(NOC):
                    xc = xin.tile([128, D], F32, tag="xc")
                    nc.sync.dma_start(xc[:], xq_e[ts(oc, 128), :])
                    wc = wink.tile([128, D], F32, tag="wc")
                    nc.sync.dma_start(wc[:], wk_e[ts(oc, 128), :])
                    for src_t, hi_t, lo_t in ((xc, xqt_hi, xqt_lo),
                                              (wc, wkt_hi, wkt_lo)):
                        for dg in range(2):
                            pt = psT.tile([128, 512], F32, tag="pt")
                            for i in range(4):
                                dc = dg * 4 + i
                                nc.tensor.transpose(pt[:, ts(i, 128)],
                                                    src_t[:, ts(dc, 128)],
                                                    ident[:])
                            copy_split(hi_t[:, dg * 4:dg * 4 + 4, ts(oc, 128)],
                                       lo_t[:, dg * 4:dg * 4 + 4, ts(oc, 128)],
                                       pt[:])

            def transpose_weight(w_ext, out_dtype, tag, wt_pool, split=False):
                """DRAM W[d, d'] -> SBUF WT[128(d'p), d'c, d] (= W.T).
                split=True returns (hi, lo) bf16 pair."""
                if split:
                    wt_hi = wt_pool.tile([128, NDC, D], BF16, tag=f"{tag}_hi")
                    wt_lo = wt_pool.tile([128, NDC, D], BF16, tag=f"{tag}_lo")
                else:
                    wt = wt_pool.tile([128, NDC, D], out_dtype, tag=tag)
                with tc.tile_pool(name=f"win_{tag}", bufs=4) as win:
                    for oc in range(NDC):
                        wc = win.tile([128, D], F32, tag="wc")
                        nc.sync.dma_start(wc[:], w_ext[ts(oc, 128), :])
                        for dg in range(2):
                            pt = psT.tile([128, 512], F32, tag="pt")
                            for i in range(4):
                                dc = dg * 4 + i
                                nc.tensor.transpose(pt[:, ts(i, 128)],
                                                    wc[:, ts(dc, 128)], ident[:])
                            sl = (slice(None), slice(dg * 4, dg * 4 + 4),
                                  ts(oc, 128))
                            if split:
                                copy_split(wt_hi[sl], wt_lo[sl], pt[:])
                            else:
                                nc.scalar.copy(wt[sl], pt[:])
                return (wt_hi, wt_lo) if split else wt

            def rsqrt_newton(ssq):
                """[128,1] fp32 1/sqrt(ssq), fp32-accurate via 2 Newton steps."""
                s1 = small.tile([128, 1], F32, tag="nw_s1")
                nc.scalar.activation(s1[:], ssq[:],
                                     mybir.ActivationFunctionType.Sqrt)
                z = small.tile([128, 1], F32, tag="nw_z")
                nc.vector.reciprocal(z[:], s1[:])
                for _ in range(2):
                    t = small.tile([128, 1], F32, tag="nw_t")
                    nc.vector.tensor_mul(t[:], z[:], z[:])
                    nc.vector.tensor_mul(t[:], t[:], ssq[:])
                    nc.vector.tensor_scalar(t[:], t[:], -0.5, 1.5,
                                            op0=mybir.AluOpType.mult,
                                            op1=mybir.AluOpType.add)
                    nc.vector.tensor_mul(z[:], z[:], t[:])
                return z

            def qk_phase(w_ext, bias, out_hi, out_lo, wtag, pre=None,
                         dram_sink=None, mid_cb=None):
                """Project own rows with W (hi/lo 3-pass bf16), l2-normalize,
                transpose into split out_hi/out_lo [128, dc, row]."""
                with tc.tile_pool(name=f"wt_{wtag}", bufs=1) as wtp, \
                     tc.tile_pool(name=f"ph_{wtag}", bufs=2) as ph:
                    if pre is None:
                        wt_hi, wt_lo = transpose_weight(w_ext, BF16, wtag, wtp,
                                                        split=True)
                    else:
                        wt_hi, wt_lo = pre
                    for oc in range(NOC):
                        if oc == 4 and mid_cb is not None:
                            mid_cb()
                        qh = ph.tile([128, D], F32, tag="qh")
                        pqs = []
                        sqa = []
                        for db in range(2):
                            pq = psA.tile([128, 512], F32, tag="mm")
                            pqs.append(pq)
                            for dc in range(NDC):
                                for li, (lh, rh) in enumerate((
                                        (xqt_hi, wt_hi), (xqt_hi, wt_lo),
                                        (xqt_lo, wt_hi))):
                                    nc.tensor.matmul(
                                        pq[:], lh[:, dc, ts(oc, 128)],
                                        rh[:, dc, ts(db, 512)],
                                        start=(dc == 0 and li == 0),
                                        stop=False)
                            nc.tensor.matmul(pq[:], ones2[:],
                                             bias[:, ts(db, 512)],
                                             start=False, stop=True)
                            sq = small.tile([128, 1], F32, tag=f"ssq{db}")
                            sqa.append(sq)
                            nc.scalar.activation(
                                qh[:, ts(db, 512)], pq[:],
                                mybir.ActivationFunctionType.Square,
                                accum_out=sq[:])
                        ssq = small.tile([128, 1], F32, tag="ssq")
                        nc.vector.tensor_add(ssq[:], sqa[0][:], sqa[1][:])
                        z = rsqrt_newton(ssq)
                        for db in range(2):
                            nc.scalar.activation(
                                qh[:, ts(db, 512)], pqs[db][:],
                                mybir.ActivationFunctionType.Copy,
                                scale=z[:])
                        for dg in range(2):
                            pt = psT.tile([128, 512], F32, tag="pt")
                            for i in range(4):
                                dc = dg * 4 + i
                                nc.tensor.transpose(pt[:, ts(i, 128)],
                                                    qh[:, ts(dc, 128)], ident[:])
                            if dram_sink is None:
                                sl = (slice(None), slice(dg * 4, dg * 4 + 4),
                                      ts(oc, 128))
                                copy_split(out_hi[sl], out_lo[sl], pt[:])
                            else:
                                st_hi = ph.tile([128, 4, 128], BF16, tag="st_hi")
                                st_lo = ph.tile([128, 4, 128], BF16, tag="st_lo")
                                copy_split(st_hi[:], st_lo[:], pt[:])
                                for hl, st in ((0, st_hi), (1, st_lo)):
                                    nc.gpsimd.dma_start(
                                        dram_sink[hl, ts(dg, 512),
                                                  ts(oc, 128)].rearrange(
                                            "(o p) j -> p o j", p=128),
                                        st[:])

            ktf_hi = ktfp.tile([128, NDC, P], BF16, tag="ktf_hi")
            wpre = {}

            def build_wq_wv():
                wpre["wq"] = transpose_weight(wq_e, BF16, "wq", wqrp, split=True)
                wpre["wv"] = transpose_weight(wv_e, BF16, "wv", wvrp)

            PHASES = os.environ.get("KERNEL_PHASES", "full")
            # ---- K first (feeds the collective), then Q, then V ----
            qk_phase(wk_e, bias_bc["k"], None, None, "wk", dram_sink=kt_bounce,
                     mid_cb=build_wq_wv, pre=(wkt_hi, wkt_lo))
            wk_scope.__exit__(None, None, None)
            wqt_hi, wqt_lo = wpre["wq"]
            wvt = wpre["wv"]
            ktflp = top.enter_context(
                tc.tile_pool(name="ktf_lo_pool", bufs=1, side="right"))
            ktf_lo = ktflp.tile([128, NDC, P], BF16, tag="ktf_lo")
            if PHASES != "qkv":
                nc.gpsimd.collective_compute(
                    "AllGather", mybir.AluOpType.bypass, replica_groups=RG,
                    ins=[kt_bounce[0]], outs=[kt_gather_hi[:]])
                nc.gpsimd.collective_compute(
                    "AllGather", mybir.AluOpType.bypass, replica_groups=RG,
                    ins=[kt_bounce[1]], outs=[kt_gather_lo[:]])

            # ---- V in bf16 ----
            with tc.tile_pool(name="phv", bufs=2) as phv:
                for oc in range(NOC):
                    vsb = phv.tile([128, D], BF16, tag="vsb")
                    for db in range(2):
                        pv = psA.tile([128, 512], F32, tag="mm")
                        for dc in range(NDC):
                            nc.tensor.matmul(pv[:], xqt_hi[:, dc, ts(oc, 128)],
                                             wvt[:, dc, ts(db, 512)],
                                             start=(dc == 0), stop=False)
                        nc.tensor.matmul(pv[:], ones2[:],
                                         bias_bc["v"][:, ts(db, 512)],
                                         start=False, stop=True)
                        nc.scalar.copy(vsb[:, ts(db, 512)], pv[:])
                    nc.gpsimd.dma_start(v_bounce[ts(oc, 128), :], vsb[:])
            if PHASES != "qkv":
                nc.gpsimd.collective_compute(
                    "AllGather", mybir.AluOpType.bypass, replica_groups=RG,
                    ins=[v_bounce[:]], outs=[v_gather[:]])

            qtp = top.enter_context(tc.tile_pool(name="qt_pool", bufs=1, side="right"))
            qt_hi = qtp.tile([128, NDC, H], BF16, tag="qt_hi")
            qt_lo = qtp.tile([128, NDC, H], BF16, tag="qt_lo")
            qk_phase(wq_e, bias_bc["q"], qt_hi, qt_lo, "wq", pre=(wqt_hi, wqt_lo))

            # ---- load gathered K^T and V ----
            wv_scope.__exit__(None, None, None)
            wq_scope.__exit__(None, None, None)
            xq_scope.__exit__(None, None, None)
            bigp = top.enter_context(tc.tile_pool(name="big_pool", bufs=1))
            vf = bigp.tile([128, 2 * NOC, D], BF16, tag="vf")
            if PHASES != "qkv":
                for g in range(2):
                    nc.sync.dma_start(
                        ktf_hi[:, :, g * H:(g + 1) * H],
                        kt_gather_hi[g].rearrange("(o p) j -> p o j", p=128))
                for g in range(2):
                    nc.sync.dma_start(
                        ktf_lo[:, :, g * H:(g + 1) * H],
                        kt_gather_lo[g].rearrange("(o p) j -> p o j", p=128))
                for g in range(2):
                    nc.gpsimd.dma_start(
                        vf[:, g * NOC:(g + 1) * NOC, :],
                        v_gather[g].rearrange("(o p) d -> p o d", p=128))

            # ---- fused scores / top8 / masked-softmax / features ----
            run_ph6 = PHASES not in ("qkv", "cc")
            with tc.tile_pool(name="sc", bufs=2) as sc, \
                 tc.tile_pool(name="dmp", bufs=1) as dmp, \
                 tc.tile_pool(name="wtrp", bufs=4) as wtrp:
                idx_all = sc.tile([128, NOC, 8], U32, tag="idx_all")
                w_all = sc.tile([128, NOC, 8], F32, tag="w_all")

                def stage_scores(pc):
                    S = sc.tile([128, P], F32, tag="S")
                    # diagonal mask, built before the matmuls finish:
                    # dm = -1e9 where j == global row id else 0
                    dm = dmp.tile([128, P], BF16, tag="dm")
                    nc.vector.tensor_scalar(dm[:], iota_f[:],
                                            rid_sb[:, pc:pc + 1], NEG_INF,
                                            op0=mybir.AluOpType.is_equal,
                                            op1=mybir.AluOpType.mult)
                    for jb in range(4):
                        ps = psA.tile([128, 512], F32, tag="mm")
                        for dc in range(NDC):
                            for li, (lh, rh) in enumerate((
                                    (qt_hi, ktf_hi), (qt_hi, ktf_lo),
                                    (qt_lo, ktf_hi))):
                                nc.tensor.matmul(
                                    ps[:], lh[:, dc, ts(pc, 128)],
                                    rh[:, dc, ts(jb, 512)],
                                    start=(dc == 0 and li == 0),
                                    stop=(dc == NDC - 1 and li == 2))
                        nc.vector.tensor_add(S[:, ts(jb, 512)], ps[:],
                                             dm[:, ts(jb, 512)])
                    # top-8 values + indices
                    vals = small.tile([128, 8], F32, tag="vals")
                    nc.vector.max(out=vals[:], in_=S[:])
                    nc.vector.max_index(out=idx_all[:, pc, :], in_max=vals[:],
                                        in_values=S[:])
                    # softmax weights over the top-8
                    negmax = small.tile([128, 1], F32, tag="negmax")
                    nc.vector.tensor_scalar_mul(negmax[:], vals[:, 0:1], -TEMP_INV)
                    e8 = small.tile([128, 8], F32, tag="e8")
                    nc.scalar.activation(e8[:], vals[:],
                                         mybir.ActivationFunctionType.Exp,
                                         bias=negmax[:], scale=TEMP_INV)
                    s8 = small.tile([128, 1], F32, tag="s8")
                    nc.vector.tensor_reduce(s8[:], e8[:], axis=mybir.AxisListType.X,
                                            op=mybir.AluOpType.add)
                    r8 = small.tile([128, 1], F32, tag="r8")
                    nc.vector.reciprocal(r8[:], s8[:])
                    nc.vector.tensor_scalar_mul(w_all[:, pc, :], e8[:], r8[:])
                    # masked exp row -> sparse weight matrix (bf16)
                    eb = sc.tile([128, P], BF16, tag="eb")
                    nc.scalar.activation(eb[:], S[:],
                                         mybir.ActivationFunctionType.Exp,
                                         bias=negmax[:], scale=TEMP_INV)
                    nc.vector.scalar_tensor_tensor(
                        out=eb[:], in0=S[:], scalar=vals[:, 7:8], in1=eb[:],
                        op0=mybir.AluOpType.is_ge, op1=mybir.AluOpType.mult)
                    # transpose W row-block
                    wtr = wtrp.tile([128, 2 * NOC, 128], BF16, tag="wtr")
                    for jg in range(2):
                        pw = psT.tile([128, 1024], BF16, tag="pw")
                        for i in range(8):
                            jc = jg * 8 + i
                            nc.tensor.transpose(pw[:, ts(i, 128)],
                                                eb[:, ts(jc, 128)], ident_bf[:])
                        nc.scalar.copy(wtr[:, jg * 8:jg * 8 + 8, :], pw[:])
                    return wtr, r8

                def stage_features(pc, wtr, r8):
                    feat = sc.tile([128, D], F32, tag="feat")
                    for db in range(2):
                        pf = psA.tile([128, 512], F32, tag="mm")
                        for jc in range(2 * NOC):
                            nc.tensor.matmul(pf[:], wtr[:, jc, :],
                                             vf[:, jc, ts(db, 512)],
                                             start=(jc == 0),
                                             stop=(jc == 2 * NOC - 1))
                        nc.scalar.activation(feat[:, ts(db, 512)], pf[:],
                                             mybir.ActivationFunctionType.Copy,
                                             scale=r8[:])
                    nc.sync.dma_start(features_e[ts(pc, 128), :], feat[:])

                SKEW = 3
                pend = []
                for pc in range(NOC if run_ph6 else 0):
                    pend.append((pc, stage_scores(pc)))
                    if len(pend) > SKEW:
                        fpc, args = pend.pop(0)
                        stage_features(fpc, *args)
                for fpc, args in pend:
                    stage_features(fpc, *args)
                if run_ph6:
                    nc.sync.dma_start(routes_e.rearrange("(o p) k -> p o k", p=128),
                                      idx_all[:])
                    nc.sync.dma_start(weights_e.rearrange("(o p) k -> p o k", p=128),
                                      w_all[:])

    nc.compile()
    return nc


def _get_nc():
    if "nc" not in _CACHE:
        _CACHE["nc"] = _build()
    return _CACHE["nc"]


def kernel(x, Wq, bq, Wk, bk, Wv, bv):
    x = np.ascontiguousarray(np.asarray(x, dtype=np.float32))
    Wq = np.ascontiguousarray(np.asarray(Wq, dtype=np.float32))
    Wk = np.ascontiguousarray(np.asarray(Wk, dtype=np.float32))
    Wv = np.ascontiguousarray(np.asarray(Wv, dtype=np.float32))
    bq = np.asarray(bq, dtype=np.float32).reshape(1, D)
    bk = np.asarray(bk, dtype=np.float32).reshape(1, D)
    bv = np.asarray(bv, dtype=np.float32).reshape(1, D)

    nc = _get_nc()
    in_maps = []
    for c in range(N_CORES):
        b, h = c // 2, c % 2
        xs = np.ascontiguousarray(x[b, 1 + h * H:1 + (h + 1) * H, :])
        rid = (h * H + np.arange(H)).astype(np.float32)
        in_maps.append({
            "xq": xs, "wq": Wq, "wk": Wk, "wv": Wv,
            "bq": bq, "bk": bk, "bv": bv, "row_ids": rid,
        })

    trace = bool(int(os.environ.get("BASS_KERNEL_TRACE", "0")))
    res = run_bass_kernel_spmd(nc, in_maps, core_ids=list(range(N_CORES)),
                               trace=trace)
    _CACHE["last_result"] = res

    routes = np.empty((B, P, 8), dtype=np.int32)
    weights = np.empty((B, P, 8), dtype=np.float32)
    features = np.empty((B, P, D), dtype=np.float32)
    for c in range(N_CORES):
        b, h = c // 2, c % 2
        sl = slice(h * H, (h + 1) * H)
        routes[b, sl] = res.results[c]["routes"].astype(np.int32)
        weights[b, sl] = res.results[c]["weights"]
        features[b, sl] = res.results[c]["features"]
    return routes, weights, features


# revision 45
# speedup vs baseline: 1.0571x; 1.0424x over previous
"""Trainium2 Bass kernel for nn_BaselineWormholeRouter.

Computes, per batch b over x[b, 1:, :]:
  q = l2norm(x @ Wq.T + bq); k = l2norm(x @ Wk.T + bk); v = x @ Wv.T + bv
  scores = q @ k.T with diag masked to -1e9
  routes, topk = top8(scores / 0.1); weights = softmax(topk)
  features = sum_k weights[k] * v[routes[k]]

Sharding: 8 cores = (batch b in 0..3) x (query-row half h in 0..1).
Each core computes q/k/v for its own 1024 rows, pair-exchanges k^T and v
via AllGather over core pairs (0,1),(2,3),(4,6),(6,7), then does the
full scores row-block + top8 + masked-softmax features locally.

Precision: Q/K projections and the scores matmul use a bf16 hi/lo
split (x ~ hi + lo, 3 accumulation passes: hh + hl + lh) giving
~2e-7-relative score accuracy at bf16 matmul throughput -- plain fp32r
or bf16 would swap too many near-tied top-8 entries (routes are
rank-exact vs the fp32 reference except a handful of ties within
~2e-7). The norm uses IEEE reciprocal + Newton-refined sqrt. V and the
features matmul run in plain bf16 (features gate is loose).
"""
import os
import sys

import numpy as np

sys.path.insert(0, "/opt/trn_rl_repo")

import concourse.bass as bass  # noqa: E402
import concourse.tile as tile  # noqa: E402
from concourse import bacc, mybir  # noqa: E402
from concourse.bass import ts  # noqa: E402
from concourse.bass_utils import run_bass_kernel_spmd  # noqa: E402
from concourse.masks import make_identity  # noqa: E402

F32 = mybir.dt.float32
BF16 = mybir.dt.bfloat16
U32 = mybir.dt.uint32

N_CORES = 8
B, P1, D = 4, 2049, 1024
P = P1 - 1          # 2048 rows after skip_first
H = P // 2          # 1024 rows per core
NEG_INF = -1e9
TEMP_INV = 10.0     # 1/temperature
RG = [[0, 1], [2, 3], [4, 5], [6, 7]]

_CACHE = {}


def _build():
    nc = bacc.Bacc("TRN2", target_bir_lowering=False, debug=False,
                   num_devices=N_CORES)

    # ---- DRAM I/O ----
    xq_e = nc.dram_tensor("xq", [H, D], F32, kind="ExternalInput").ap()
    wq_e = nc.dram_tensor("wq", [D, D], F32, kind="ExternalInput").ap()
    wk_e = nc.dram_tensor("wk", [D, D], F32, kind="ExternalInput").ap()
    wv_e = nc.dram_tensor("wv", [D, D], F32, kind="ExternalInput").ap()
    bq_e = nc.dram_tensor("bq", [1, D], F32, kind="ExternalInput").ap()
    bk_e = nc.dram_tensor("bk", [1, D], F32, kind="ExternalInput").ap()
    bv_e = nc.dram_tensor("bv", [1, D], F32, kind="ExternalInput").ap()
    rid_e = nc.dram_tensor("row_ids", [H], F32, kind="ExternalInput").ap()

    routes_e = nc.dram_tensor("routes", [H, 8], U32, kind="ExternalOutput").ap()
    weights_e = nc.dram_tensor("weights", [H, 8], F32, kind="ExternalOutput").ap()
    features_e = nc.dram_tensor("features", [H, D], F32, kind="ExternalOutput").ap()

    kt_bounce = nc.dram_tensor("kt_bounce", [2, D, H], BF16).ap()
    v_bounce = nc.dram_tensor("v_bounce", [H, D], BF16).ap()
    kt_gather_hi = nc.dram_tensor("kt_gather_hi", [2, D, H], BF16).ap()
    kt_gather_lo = nc.dram_tensor("kt_gather_lo", [2, D, H], BF16).ap()
    v_gather = nc.dram_tensor("v_gather", [2, H, D], BF16).ap()

    NOC = H // 128   # 8 row chunks per core
    NDC = D // 128   # 8 contraction chunks

    with tile.TileContext(nc) as tc:
        from contextlib import ExitStack
        with ExitStack() as top:
            cpool = top.enter_context(tc.tile_pool(name="consts", bufs=1))
            small = top.enter_context(tc.tile_pool(name="small", bufs=6))
            ktfp = top.enter_context(tc.tile_pool(name="ktf_pool", bufs=1))
            psA = top.enter_context(tc.tile_pool(name="psA", bufs=4, space="PSUM"))
            psT = top.enter_context(tc.tile_pool(name="psT", bufs=2, space="PSUM"))
            xq_scope = tc.tile_pool(name="xqt_pool", bufs=1)
            xq_pool = xq_scope.__enter__()

            # ---- constants ----
            ident = cpool.tile([128, 128], F32, tag="ident")
            make_identity(nc, ident[:])
            ident_bf = cpool.tile([128, 128], BF16, tag="ident_bf")
            nc.vector.tensor_copy(ident_bf[:], ident[:])
            iota_f = cpool.tile([128, P], F32, tag="iota")
            nc.gpsimd.iota(iota_f[:], pattern=[[1, P]], base=0,
                           channel_multiplier=0,
                           allow_small_or_imprecise_dtypes=True)
            warm = cpool.tile([128, 512], BF16, tag="warm")
            nc.vector.memset(warm[:], 0.0)
            for wi in range(16):
                pwm = psA.tile([128, 512], F32, tag="mm")
                nc.tensor.matmul(pwm[:], warm[:, 0:128], warm[:],
                                 start=True, stop=True)
                nc.scalar.copy(warm[0:1, 0:1], pwm[0:1, 0:1])

            rid_sb = cpool.tile([128, NOC], F32, tag="rid")
            nc.sync.dma_start(rid_sb[:], rid_e.rearrange("(o p) -> p o", p=128))
            ones_r = cpool.tile([1, 128], F32, tag="ones")
            nc.vector.memset(ones_r[:], 1.0)

            # bias rows as bf16 hi/lo pairs; added inside each matmul
            # accumulation via a K=2 ones matmul.
            ones2 = cpool.tile([2, 128], BF16, tag="ones2")
            nc.vector.memset(ones2[:], 1.0)
            bias_bc = {}
            with tc.tile_pool(name="browp", bufs=2) as browp:
                for nm, be in (("q", bq_e), ("k", bk_e), ("v", bv_e)):
                    brow = browp.tile([1, D], F32, tag="brow")
                    nc.sync.dma_start(brow[:], be)
                    bh = browp.tile([1, D], BF16, tag="bh")
                    nc.scalar.copy(bh[:], brow[:])
                    bl = browp.tile([1, D], BF16, tag="bl")
                    nc.vector.tensor_sub(bl[:], brow[:], bh[:])
                    b2 = cpool.tile([2, D], BF16, tag=f"bias_{nm}")
                    nc.sync.dma_start(b2[0:1, :], bh[:])
                    nc.sync.dma_start(b2[1:2, :], bl[:])
                    bias_bc[nm] = b2

            def copy_split(dst_hi, dst_lo, psum):
                """PSUM f32 -> bf16 hi + bf16 residual lo."""
                nc.scalar.copy(dst_hi, psum)
                nc.vector.tensor_sub(dst_lo, psum, dst_hi)

            wq_scope = tc.tile_pool(name="wt_wq", bufs=1)
            wqrp = wq_scope.__enter__()
            wv_scope = tc.tile_pool(name="wt_wv", bufs=1)
            wvrp = wv_scope.__enter__()

            # ---- interleaved build: XQT hi/lo and WkT hi/lo ----
            # x chunks and wk chunks stream on the same sync queue but gate
            # different transpose groups, so the PE always has one ready.
            wk_scope = tc.tile_pool(name="wt_wk_pre", bufs=1)
            wkpp = wk_scope.__enter__()
            xqt_hi = xq_pool.tile([128, NDC, H], BF16, tag="xqt_hi")
            xqt_lo = xq_pool.tile([128, NDC, H], BF16, tag="xqt_lo")
            wkt_hi = wkpp.tile([128, NDC, D], BF16, tag="wk_hi")
            wkt_lo = wkpp.tile([128, NDC, D], BF16, tag="wk_lo")
            with tc.tile_pool(name="xin", bufs=2) as xin, \
                 tc.tile_pool(name="wink", bufs=2) as wink:
                for oc in range(NOC):
                    xc = xin.tile([128, D], F32, tag="xc")
                    nc.sync.dma_start(xc[:], xq_e[ts(oc, 128), :])
                    wc = wink.tile([128, D], F32, tag="wc")
                    nc.sync.dma_start(wc[:], wk_e[ts(oc, 128), :])
                    for src_t, hi_t, lo_t in ((xc, xqt_hi, xqt_lo),
                                              (wc, wkt_hi, wkt_lo)):
                        for dg in range(2):
                            pt = psT.tile([128, 512], F32, tag="pt")
                            for i in range(4):
                                dc = dg * 4 + i
                                nc.tensor.transpose(pt[:, ts(i, 128)],
                                                    src_t[:, ts(dc, 128)],
                                                    ident[:])
                            copy_split(hi_t[:, dg * 4:dg * 4 + 4, ts(oc, 128)],
                                       lo_t[:, dg * 4:dg * 4 + 4, ts(oc, 128)],
                                       pt[:])

            def transpose_weight(w_ext, out_dtype, tag, wt_pool, split=False):
                """DRAM W[d, d'] -> SBUF WT[128(d'p), d'c, d] (= W.T).
                split=True returns (hi, lo) bf16 pair."""
                if split:
                    wt_hi = wt_pool.tile([128, NDC, D], BF16, tag=f"{tag}_hi")
                    wt_lo = wt_pool.tile([128, NDC, D], BF16, tag=f"{tag}_lo")
                else:
                    wt = wt_pool.tile([128, NDC, D], out_dtype, tag=tag)
                with tc.tile_pool(name=f"win_{tag}", bufs=4) as win:
                    for oc in range(NDC):
                        wc = win.tile([128, D], F32, tag="wc")
                        nc.sync.dma_start(wc[:], w_ext[ts(oc, 128), :])
                        for dg in range(2):
                            pt = psT.tile([128, 512], F32, tag="pt")
                            for i in range(4):
                                dc = dg * 4 + i
                                nc.tensor.transpose(pt[:, ts(i, 128)],
                                                    wc[:, ts(dc, 128)], ident[:])
                            sl = (slice(None), slice(dg * 4, dg * 4 + 4),
                                  ts(oc, 128))
                            if split:
                                copy_split(wt_hi[sl], wt_lo[sl], pt[:])
                            else:
                                nc.scalar.copy(wt[sl], pt[:])
                return (wt_hi, wt_lo) if split else wt

            def rsqrt_newton(ssq):
                """[128,1] fp32 1/sqrt(ssq), fp32-accurate via 2 Newton steps."""
                s1 = small.tile([128, 1], F32, tag="nw_s1")
                nc.scalar.activation(s1[:], ssq[:],
                                     mybir.ActivationFunctionType.Sqrt)
                z = small.tile([128, 1], F32, tag="nw_z")
                nc.vector.reciprocal(z[:], s1[:])
                for _ in range(2):
                    t = small.tile([128, 1], F32, tag="nw_t")
                    nc.vector.tensor_mul(t[:], z[:], z[:])
                    nc.vector.tensor_mul(t[:], t[:], ssq[:])
                    nc.vector.tensor_scalar(t[:], t[:], -0.5, 1.5,
                                            op0=mybir.AluOpType.mult,
                                            op1=mybir.AluOpType.add)
                    nc.vector.tensor_mul(z[:], z[:], t[:])
                return z

            def qk_phase(w_ext, bias, out_hi, out_lo, wtag, pre=None,
                         dram_sink=None, mid_cb=None):
                """Project own rows with W (hi/lo 3-pass bf16), l2-normalize,
                transpose into split out_hi/out_lo [128, dc, row]."""
                with tc.tile_pool(name=f"wt_{wtag}", bufs=1) as wtp, \
                     tc.tile_pool(name=f"ph_{wtag}", bufs=2) as ph:
                    if pre is None:
                        wt_hi, wt_lo = transpose_weight(w_ext, BF16, wtag, wtp,
                                                        split=True)
                    else:
                        wt_hi, wt_lo = pre
                    def proj_stage_a(oc):
                        qh = ph.tile([128, D], F32, tag="qh")
                        pqs = []
                        sqa = []
                        for db in range(2):
                            pq = psA.tile([128, 512], F32, tag="mm")
                            pqs.append(pq)
                            for dc in range(NDC):
                                for li, (lh, rh) in enumerate((
                                        (xqt_hi, wt_hi), (xqt_hi, wt_lo),
                                        (xqt_lo, wt_hi))):
                                    nc.tensor.matmul(
                                        pq[:], lh[:, dc, ts(oc, 128)],
                                        rh[:, dc, ts(db, 512)],
                                        start=(dc == 0 and li == 0),
                                        stop=False)
                            nc.tensor.matmul(pq[:], ones2[:],
                                             bias[:, ts(db, 512)],
                                             start=False, stop=True)
                            sq = small.tile([128, 1], F32, tag=f"ssq{db}")
                            sqa.append(sq)
                            nc.scalar.activation(
                                qh[:, ts(db, 512)], pq[:],
                                mybir.ActivationFunctionType.Square,
                                accum_out=sq[:])
                        ssq = small.tile([128, 1], F32, tag="ssq")
                        nc.vector.tensor_add(ssq[:], sqa[0][:], sqa[1][:])
                        z = rsqrt_newton(ssq)
                        for db in range(2):
                            nc.scalar.activation(
                                qh[:, ts(db, 512)], pqs[db][:],
                                mybir.ActivationFunctionType.Copy,
                                scale=z[:])
                        return oc, qh

                    def proj_stage_b(oc, qh):
                        for dg in range(2):
                            pt = psT.tile([128, 512], F32, tag="pt")
                            for i in range(4):
                                dc = dg * 4 + i
                                nc.tensor.transpose(pt[:, ts(i, 128)],
                                                    qh[:, ts(dc, 128)], ident[:])
                            if dram_sink is None:
                                sl = (slice(None), slice(dg * 4, dg * 4 + 4),
                                      ts(oc, 128))
                                copy_split(out_hi[sl], out_lo[sl], pt[:])
                            else:
                                st_hi = ph.tile([128, 4, 128], BF16, tag="st_hi")
                                st_lo = ph.tile([128, 4, 128], BF16, tag="st_lo")
                                copy_split(st_hi[:], st_lo[:], pt[:])
                                for hl, st in ((0, st_hi), (1, st_lo)):
                                    nc.gpsimd.dma_start(
                                        dram_sink[hl, ts(dg, 512),
                                                  ts(oc, 128)].rearrange(
                                            "(o p) j -> p o j", p=128),
                                        st[:])

                    pend = None
                    for oc in range(NOC):
                        if oc == 4 and mid_cb is not None:
                            mid_cb()
                        cur = proj_stage_a(oc)
                        if pend is not None:
                            proj_stage_b(*pend)
                        pend = cur
                    proj_stage_b(*pend)

            ktf_hi = ktfp.tile([128, NDC, P], BF16, tag="ktf_hi")
            wpre = {}

            def build_wq_wv():
                wpre["wq"] = transpose_weight(wq_e, BF16, "wq", wqrp, split=True)
                wpre["wv"] = transpose_weight(wv_e, BF16, "wv", wvrp)

            PHASES = os.environ.get("KERNEL_PHASES", "full")
            # ---- K first (feeds the collective), then Q, then V ----
            qk_phase(wk_e, bias_bc["k"], None, None, "wk", dram_sink=kt_bounce,
                     mid_cb=build_wq_wv, pre=(wkt_hi, wkt_lo))
            wk_scope.__exit__(None, None, None)
            wqt_hi, wqt_lo = wpre["wq"]
            wvt = wpre["wv"]
            ktflp = top.enter_context(
                tc.tile_pool(name="ktf_lo_pool", bufs=1, side="right"))
            ktf_lo = ktflp.tile([128, NDC, P], BF16, tag="ktf_lo")
            if PHASES != "qkv":
                nc.gpsimd.collective_compute(
                    "AllGather", mybir.AluOpType.bypass, replica_groups=RG,
                    ins=[kt_bounce[0]], outs=[kt_gather_hi[:]])
                nc.gpsimd.collective_compute(
                    "AllGather", mybir.AluOpType.bypass, replica_groups=RG,
                    ins=[kt_bounce[1]], outs=[kt_gather_lo[:]])

            # ---- V in bf16 ----
            with tc.tile_pool(name="phv", bufs=2) as phv:
                for oc in range(NOC):
                    vsb = phv.tile([128, D], BF16, tag="vsb")
                    for db in range(2):
                        pv = psA.tile([128, 512], F32, tag="mm")
                        for dc in range(NDC):
                            nc.tensor.matmul(pv[:], xqt_hi[:, dc, ts(oc, 128)],
                                             wvt[:, dc, ts(db, 512)],
                                             start=(dc == 0), stop=False)
                        nc.tensor.matmul(pv[:], ones2[:],
                                         bias_bc["v"][:, ts(db, 512)],
                                         start=False, stop=True)
                        nc.scalar.copy(vsb[:, ts(db, 512)], pv[:])
                    nc.gpsimd.dma_start(v_bounce[ts(oc, 128), :], vsb[:])
            if PHASES != "qkv":
                nc.gpsimd.collective_compute(
                    "AllGather", mybir.AluOpType.bypass, replica_groups=RG,
                    ins=[v_bounce[:]], outs=[v_gather[:]])

            qtp = top.enter_context(tc.tile_pool(name="qt_pool", bufs=1, side="right"))
            qt_hi = qtp.tile([128, NDC, H], BF16, tag="qt_hi")
            qt_lo = qtp.tile([128, NDC, H], BF16, tag="qt_lo")
            qk_phase(wq_e, bias_bc["q"], qt_hi, qt_lo, "wq", pre=(wqt_hi, wqt_lo))

            # ---- load gathered K^T and V ----
            wv_scope.__exit__(None, None, None)
            wq_scope.__exit__(None, None, None)
            xq_scope.__exit__(None, None, None)
            bigp = top.enter_context(tc.tile_pool(name="big_pool", bufs=1))
            vf = bigp.tile([128, 2 * NOC, D], BF16, tag="vf")
            if PHASES != "qkv":
                for g in range(2):
                    nc.sync.dma_start(
                        ktf_hi[:, :, g * H:(g + 1) * H],
                        kt_gather_hi[g].rearrange("(o p) j -> p o j", p=128))
                for g in range(2):
                    nc.sync.dma_start(
                        ktf_lo[:, :, g * H:(g + 1) * H],
                        kt_gather_lo[g].rearrange("(o p) j -> p o j", p=128))
                for g in range(2):
                    nc.gpsimd.dma_start(
                        vf[:, g * NOC:(g + 1) * NOC, :],
                        v_gather[g].rearrange("(o p) d -> p o d", p=128))

            # ---- fused scores / top8 / masked-softmax / features ----
            run_ph6 = PHASES not in ("qkv", "cc")
            with tc.tile_pool(name="sc", bufs=2) as sc, \
                 tc.tile_pool(name="dmp", bufs=1) as dmp, \
                 tc.tile_pool(name="wtrp", bufs=4) as wtrp:
                idx_all = sc.tile([128, NOC, 8], U32, tag="idx_all")
                w_all = sc.tile([128, NOC, 8], F32, tag="w_all")

                def stage_scores(pc):
                    S = sc.tile([128, P], F32, tag="S")
                    # diagonal mask, built before the matmuls finish:
                    # dm = -1e9 where j == global row id else 0
                    dm = dmp.tile([128, P], BF16, tag="dm")
                    nc.vector.tensor_scalar(dm[:], iota_f[:],
                                            rid_sb[:, pc:pc + 1], NEG_INF,
                                            op0=mybir.AluOpType.is_equal,
                                            op1=mybir.AluOpType.mult)
                    for jb in range(4):
                        ps = psA.tile([128, 512], F32, tag="mm")
                        for dc in range(NDC):
                            for li, (lh, rh) in enumerate((
                                    (qt_hi, ktf_hi), (qt_hi, ktf_lo),
                                    (qt_lo, ktf_hi))):
                                nc.tensor.matmul(
                                    ps[:], lh[:, dc, ts(pc, 128)],
                                    rh[:, dc, ts(jb, 512)],
                                    start=(dc == 0 and li == 0),
                                    stop=(dc == NDC - 1 and li == 2))
                        nc.vector.tensor_add(S[:, ts(jb, 512)], ps[:],
                                             dm[:, ts(jb, 512)])
                    # top-8 values + indices
                    vals = small.tile([128, 8], F32, tag="vals")
                    nc.vector.max(out=vals[:], in_=S[:])
                    nc.vector.max_index(out=idx_all[:, pc, :], in_max=vals[:],
                                        in_values=S[:])
                    # softmax weights over the top-8
                    negmax = small.tile([128, 1], F32, tag="negmax")
                    nc.vector.tensor_scalar_mul(negmax[:], vals[:, 0:1], -TEMP_INV)
                    e8 = small.tile([128, 8], F32, tag="e8")
                    nc.scalar.activation(e8[:], vals[:],
                                         mybir.ActivationFunctionType.Exp,
                                         bias=negmax[:], scale=TEMP_INV)
                    s8 = small.tile([128, 1], F32, tag="s8")
                    nc.vector.tensor_reduce(s8[:], e8[:], axis=mybir.AxisListType.X,
                                            op=mybir.AluOpType.add)
                    r8 = small.tile([128, 1], F32, tag="r8")
                    nc.vector.reciprocal(r8[:], s8[:])
                    nc.vector.tensor_scalar_mul(w_all[:, pc, :], e8[:], r8[:])
                    # masked exp row -> sparse weight matrix (bf16)
                    eb = sc.tile([128, P], BF16, tag="eb")
                    nc.scalar.activation(eb[:], S[:],
                                         mybir.ActivationFunctionType.Exp,
                                         bias=negmax[:], scale=TEMP_INV)
                    nc.vector.scalar_tensor_tensor(
                        out=eb[:], in0=S[:], scalar=vals[:, 7:8], in1=eb[:],
                        op0=mybir.AluOpType.is_ge, op1=mybir.AluOpType.mult)
                    # transpose W row-block
                    wtr = wtrp.tile([128, 2 * NOC, 128], BF16, tag="wtr")
                    for jg in range(2):
                        pw = psT.tile([128, 1024], BF16, tag="pw")
                        for i in range(8):
                            jc = jg * 8 + i
                            nc.tensor.transpose(pw[:, ts(i, 128)],
                                                eb[:, ts(jc, 128)], ident_bf[:])
                        nc.scalar.copy(wtr[:, jg * 8:jg * 8 + 8, :], pw[:])
                    return wtr, r8

                def stage_features(pc, wtr, r8):
                    feat = sc.tile([128, D], F32, tag="feat")
                    for db in range(2):
                        pf = psA.tile([128, 512], F32, tag="mm")
                        for jc in range(2 * NOC):
                            nc.tensor.matmul(pf[:], wtr[:, jc, :],
                                             vf[:, jc, ts(db, 512)],
                                             start=(jc == 0),
                                             stop=(jc == 2 * NOC - 1))
                        nc.scalar.activation(feat[:, ts(db, 512)], pf[:],
                                             mybir.ActivationFunctionType.Copy,
                                             scale=r8[:])
                    nc.sync.dma_start(features_e[ts(pc, 128), :], feat[:])

                SKEW = 3
                pend = []
                for pc in range(NOC if run_ph6 else 0):
                    pend.append((pc, stage_scores(pc)))
                    if len(pend) > SKEW:
                        fpc, args = pend.pop(0)
                        stage_features(fpc, *args)
                for fpc, args in pend:
                    stage_features(fpc, *args)
                if run_ph6:
                    nc.sync.dma_start(routes_e.rearrange("(o p) k -> p o k", p=128),
                                      idx_all[:])
                    nc.sync.dma_start(weights_e.rearrange("(o p) k -> p o k", p=128),
                                      w_all[:])

    nc.compile()
    return nc


def _get_nc():
    if "nc" not in _CACHE:
        _CACHE["nc"] = _build()
    return _CACHE["nc"]


def kernel(x, Wq, bq, Wk, bk, Wv, bv):
    x = np.ascontiguousarray(np.asarray(x, dtype=np.float32))
    Wq = np.ascontiguousarray(np.asarray(Wq, dtype=np.float32))
    Wk = np.ascontiguousarray(np.asarray(Wk, dtype=np.float32))
    Wv = np.ascontiguousarray(np.asarray(Wv, dtype=np.float32))
    bq = np.asarray(bq, dtype=np.float32).reshape(1, D)
    bk = np.asarray(bk, dtype=np.float32).reshape(1, D)
    bv = np.asarray(bv, dtype=np.float32).reshape(1, D)

    nc = _get_nc()
    in_maps = []
    for c in range(N_CORES):
        b, h = c // 2, c % 2
        xs = np.ascontiguousarray(x[b, 1 + h * H:1 + (h + 1) * H, :])
        rid = (h * H + np.arange(H)).astype(np.float32)
        in_maps.append({
            "xq": xs, "wq": Wq, "wk": Wk, "wv": Wv,
            "bq": bq, "bk": bk, "bv": bv, "row_ids": rid,
        })

    trace = bool(int(os.environ.get("BASS_KERNEL_TRACE", "0")))
    res = run_bass_kernel_spmd(nc, in_maps, core_ids=list(range(N_CORES)),
                               trace=trace)
    _CACHE["last_result"] = res

    routes = np.empty((B, P, 8), dtype=np.int32)
    weights = np.empty((B, P, 8), dtype=np.float32)
    features = np.empty((B, P, D), dtype=np.float32)
    for c in range(N_CORES):
        b, h = c // 2, c % 2
        sl = slice(h * H, (h + 1) * H)
        routes[b, sl] = res.results[c]["routes"].astype(np.int32)
        weights[b, sl] = res.results[c]["weights"]
        features[b, sl] = res.results[c]["features"]
    return routes, weights, features
